# revision 18
# baseline (speedup 1.0000x reference)
"""Trainium2 Bass kernel for nn_EventProjector (contrastive event loss).

Reference math:
    seq_p = sequence_output @ W.T + b ; q_p = q_event_output @ W.T + b
    x[b]  = q_p[b, mask_pos[b]]                  (single <mask> per row)
    ys    = seq_p[:, offsets, :]                 [B, L, H]
    cos   = <x, ys> / max(|x||ys|, 1e-8) ; e = exp(cos)
    loss  = mean_b( -log( sum_l e*lab / sum_l e*ev ) )

Only the L=128 shared offset rows plus one mask row per example are ever
used, and the projection is linear, so gather rows first and project
[B*L, H] instead of [B, S, H] -- ~16x less matmul work, ~25x less HBM.

Sharding: data-parallel over B across 8 cores (2 examples/core).  The
device does the heavy part: P = RT^T @ W^T (K=1024, 8x128 accumulation)
and |P_row|^2 = sum_o P^2 per row (ACT square+accumulate / DVE reduce).
The host does index gathers/transposes, the 16-row anchor projection,
the two tiny per-row dot columns (s_r.v_e, s_r.W^T b -- 8 MFLOP total),
and the final cos/exp/log tail over 2*128 scalars per core.

Perf notes (from neuron-profile traces):
  - fused fp32 matmul = 4 cyc/row and one LDWEIGHTS wait slot -> pack
    all matmul operands in ONE dram tensor so each chunk is one DMA
  - bf16 halves HBM bytes and runs 1 cyc/row; loss rel-err ~2e-6
  - HWDGE DMA issue costs ~0.6us/instruction on the issuing queue ->
    split chunk loads across the two HWDGE queues (sync + scalar)
  - PE HAM runs 1.2 GHz for the first ~3.4us -> warm it with junk
    matmuls while the first DMA chunk is still in flight
"""

import os

import numpy as np

# ---------------------------------------------------------------- config
B, S, H, L = 16, 2048, 1024, 128
NCORES = 8
PB = B // NCORES          # examples per core (2)
R = PB * L                # y rows per core (256)
KC = H // 128             # contraction chunks (8)
WRC = R + H               # packed operand columns [rt | W^T]
MASK_TOKEN_ID = 50264
EPS = 1e-8
NWARM = int(os.environ.get("KERNEL_NWARM", "10"))

# matmul operand precision: "f32" (exact, 4 cyc/row), "f32r" (1 cyc/row),
# "bf16" (1 cyc/row, half the DMA traffic)
MM_DT = os.environ.get("KERNEL_MM_DT", "bf16")
TRACE = False             # set True by test.py to profile
LAST_RESULTS = None       # BassKernelResults of the last run (for test.py)

_NC_CACHE = {}


def _build_bass(mm_dt: str):
    import concourse.bass as bass
    import concourse.bacc as bacc
    import concourse.mybir as mybir
    from concourse.tile import TileContext

    f32 = mybir.dt.float32
    if mm_dt == "bf16":
        ddt = mybir.dt.bfloat16
    elif mm_dt == "f32r":
        ddt = mybir.dt.float32r
    else:
        ddt = f32
    A = mybir.AluOpType
    AF = mybir.ActivationFunctionType
    ts = bass.ts

    nc = bacc.Bacc("TRN2", target_bir_lowering=False)

    # packed per-core operands: cols [rt(R) | W^T(H)] so every matmul's
    # operands come from a single DMA (single semaphore wait per matmul)
    wr = nc.dram_tensor("wr", [H, WRC], ddt, kind="ExternalInput")
    out_d = nc.dram_tensor("out", [128, PB], f32, kind="ExternalOutput")

    with TileContext(nc) as tc:
        with (
            tc.tile_pool(name="consts", bufs=1) as consts,
            tc.tile_pool(name="wpool", bufs=8) as wpool,
            tc.tile_pool(name="epool", bufs=2) as epool,
            tc.tile_pool(name="ppool", bufs=1, space="PSUM") as ppool,
        ):
            out_sb = consts.tile([128, PB], f32)

            # PE warm-up: HAM gates the PE to 1.2 GHz until it has seen
            # ~3.4us of activity; burn that window on junk matmuls while
            # the first wr chunk is still in flight.
            junk_l = consts.tile([128, 128], ddt)
            junk_r = consts.tile([128, 512], ddt)
            nc.gpsimd.memset(junk_l, 0)
            nc.vector.memset(junk_r, 0)
            if NWARM:
                junk_p = ppool.tile([128, 512], f32, tag="J")
                for _ in range(NWARM):
                    nc.tensor.matmul(junk_p, junk_l, junk_r,
                                     start=True, stop=True)

            # ---- projection: P[r, o] accumulated over 8 K-chunks
            pa = [ppool.tile([128, 512], f32, tag=f"A{t}", name=f"pa{t}")
                  for t in range(PB)]
            pb = [ppool.tile([128, 512], f32, tag=f"B{t}", name=f"pb{t}")
                  for t in range(PB)]
            for c in range(KC):
                wr_sb = wpool.tile([128, WRC], ddt)
                # split chunk loads over both HWDGE queues: each DMA costs
                # ~0.6us of issue time on its queue
                dma_eng = nc.sync if c < KC // 2 else nc.scalar
                dma_eng.dma_start(out=wr_sb, in_=wr[ts(c, 128), :])
                st, sp = (c == 0), (c == KC - 1)
                for t in range(PB):
                    lhsT = wr_sb[:, ts(t, 128)]
                    nc.tensor.matmul(pa[t], lhsT, wr_sb[:, R:R + 512],
                                     start=st, stop=sp)
                    nc.tensor.matmul(pb[t], lhsT, wr_sb[:, R + 512:R + 1024],
                                     start=st, stop=sp)

            # ---- per-example row norms: ACT handles bank A, DVE bank B
            # (in parallel; cos/exp/log over 2x128 scalars happen on host)
            for t in range(PB):
                scr_a = epool.tile([128, 512], f32)
                part_a = epool.tile([128, 1], f32)
                nc.scalar.activation(out=scr_a, in_=pa[t], func=AF.Square,
                                     accum_out=part_a)
                part_b = epool.tile([128, 1], f32)
                # NOTE: vector.tensor_tensor_reduce here crashes the exec
                # unit on TRN2 hardware (NRT_EXEC_UNIT_UNRECOVERABLE) --
                # both banks go through ACT square+accumulate instead.
                scr_b = epool.tile([128, 512], f32)
                nc.scalar.activation(out=scr_b, in_=pb[t], func=AF.Square,
                                     accum_out=part_b)
                nc.vector.tensor_add(out_sb[:, t:t + 1], part_a, part_b)

            nc.scalar.dma_start(out=out_d[:, :], in_=out_sb)

    nc.compile()
    return nc


def _get_nc(mm_dt: str):
    if mm_dt not in _NC_CACHE:
        _NC_CACHE[mm_dt] = _build_bass(mm_dt)
    return _NC_CACHE[mm_dt]


def _host_prep(input_ids, q_event_output, sequence_output, events, labels,
               offsets, lengths, W, b, mm_dt):
    import ml_dtypes

    ids = np.asarray(input_ids)
    q = np.asarray(q_event_output, dtype=np.float32)
    s = np.asarray(sequence_output, dtype=np.float32)
    Wf = np.asarray(W, dtype=np.float32)
    bf = np.asarray(b, dtype=np.float32)
    off = np.asarray(offsets).astype(np.int64)
    lab = np.asarray(labels).reshape(B, L).astype(np.float32)
    ev = np.asarray(events).reshape(B, L).astype(np.float32)

    mask_pos = (ids == MASK_TOKEN_ID).argmax(axis=1)            # [B]
    x = q[np.arange(B), mask_pos] @ Wf.T + bf                   # [B, H]
    xn = np.linalg.norm(x.astype(np.float64), axis=1).astype(np.float32)
    V = x @ Wf                                                  # [B, H] W^T x_e
    cvec = x @ bf                                               # [B]
    wb = bf @ Wf                                                # [H]   W^T b
    bb = np.float32(bf @ bf)

    WT = np.ascontiguousarray(Wf.T)                             # [H, H]
    Y = s[:, off, :]                                            # [B, L, H]
    # tiny per-row dot columns (vs the 0.5 GFLOP/core projection)
    dotc = np.einsum("blh,bh->bl", Y, V)                        # [B, L]
    wbc = Y @ wb                                                # [B, L]

    if mm_dt == "bf16":
        ddt = ml_dtypes.bfloat16
    else:
        ddt = np.float32
    WTd = WT.astype(ddt)

    in_maps = []
    aux = {"xn": xn, "c": cvec, "bb": bb, "lab": lab, "ev": ev,
           "dotc": dotc, "wbc": wbc}
    for i in range(NCORES):
        e0 = PB * i
        rt_i = Y[e0:e0 + PB].reshape(R, H).T                    # [H, R]
        wr_i = np.concatenate([rt_i.astype(ddt), WTd], axis=1)  # [H, R+H]
        in_maps.append({"wr": np.ascontiguousarray(wr_i)})
    return in_maps, aux


def kernel(**inputs) -> np.ndarray:
    global LAST_RESULTS
    from concourse.bass_utils import run_bass_kernel_spmd

    in_maps, aux = _host_prep(mm_dt=MM_DT, **inputs)
    nc = _get_nc(MM_DT)
    res = run_bass_kernel_spmd(nc, in_maps, core_ids=list(range(NCORES)),
                               trace=TRACE)
    LAST_RESULTS = res

    losses = []
    for i in range(NCORES):
        raw = res.results[i]["out"].astype(np.float32)          # [128, PB]
        for t in range(PB):
            e = PB * i + t
            ysq = raw[:, t] + 2.0 * aux["wbc"][e] + aux["bb"]
            dot = aux["dotc"][e] + aux["c"][e]
            cos = dot / np.maximum(np.sqrt(ysq) * aux["xn"][e], EPS)
            ee = np.exp(cos)
            num = (ee * aux["lab"][e]).sum()
            den = (ee * aux["ev"][e]).sum()
            losses.append(np.log(den) - np.log(num))
    return np.asarray(np.float32(np.mean(losses)))
